# revision 5
# baseline (speedup 1.0000x reference)
"""DigitCaps (CapsNet dynamic routing) Trainium2 kernel.

Strategy: shard the I=1152 input capsules across the 8 cores (144 each).
Each core computes its u_hat shard [256, 10, 144, 16] with the tensor
engine (block-diagonal weight packing so K=32, M=128) and keeps it
resident in SBUF as bf16, laid out [p=batch(128), free=(i, o, d)] in two
batch chunks.  The three routing iterations then run on-chip; the only
cross-core data is the i-sum s [256, 10, 16], AllReduced per iteration
(softmax over o is pointwise in i, so everything else stays local).
"""

import numpy as np

B, O, I, DO, DI = 256, 10, 1152, 16, 8
N_CORES = 8
ISH = I // N_CORES          # 144 i's per core
NQ = ISH // 16              # 9 sixteen-i blocks (= xT blocks = quad groups)
BP = 128                    # batch partition chunk
NCH = B // BP               # 2 chunks
OD = O * DO                 # 160

_cached = {}


def _build():
    import concourse.mybir as mybir
    import concourse.tile as tile
    from concourse import bacc

    f32 = mybir.dt.float32
    bf16 = mybir.dt.bfloat16
    Alu = mybir.AluOpType
    Act = mybir.ActivationFunctionType
    X = mybir.AxisListType.X

    nc = bacc.Bacc("TRN2", target_bir_lowering=False, debug=False,
                   num_devices=N_CORES)

    # Per-core inputs (pre-arranged on host):
    # xT:  [NQ, 128, B]   rows = (i16, di8) for this 16-i block, cols = batch
    # Wbd: [NQ, 128, 640] 4 groups stacked; group sub's rows [32s,32s+32) hold
    #                     its block-diagonal [(j4,di8) x (j4,o,d)] weights
    # Wk:  [NQ, 128, 160] same stacking, dense [(j4,di8) x (o,d)] (for s1)
    xT_d = nc.dram_tensor("xT", [NQ, 128, B], bf16, kind="ExternalInput")
    Wbd_d = nc.dram_tensor("Wbd", [NQ, 128, 4 * OD], bf16, kind="ExternalInput")
    Wk_d = nc.dram_tensor("Wk", [NQ, 128, OD], bf16, kind="ExternalInput")
    y_d = nc.dram_tensor("y", [B, O, DO], f32, kind="ExternalOutput")

    with tile.TileContext(nc) as tc:
        with (
            tc.tile_pool(name="weights", bufs=1) as wpool,
            tc.tile_pool(name="uhat", bufs=1) as upool,
            tc.tile_pool(name="state", bufs=1) as stpool,
            tc.tile_pool(name="tmp", bufs=3) as tmppool,
            tc.tile_pool(name="small", bufs=2) as small,
            tc.tile_pool(name="psum_u", bufs=2, space="PSUM") as psum_u,
            tc.tile_pool(name="psum_s", bufs=2, space="PSUM") as psum_s,
            tc.tile_pool(name="dram", bufs=1, space="DRAM") as dram,
        ):
            # ---- load inputs ------------------------------------------------
            xT, Wbd, Wk = [], [], []
            for q in range(NQ):
                t = wpool.tile([128, B], bf16, name=f"xT{q}")
                nc.sync.dma_start(t[:], xT_d[q])
                xT.append(t)
                t = wpool.tile([128, 4 * OD], bf16, name=f"Wbd{q}")
                nc.sync.dma_start(t[:], Wbd_d[q])
                Wbd.append(t)
                t = wpool.tile([128, OD], bf16, name=f"Wk{q}")
                nc.sync.dma_start(t[:], Wk_d[q])
                Wk.append(t)

            # persistent per-chunk state
            u = [upool.tile([128, ISH, O, DO], bf16, name=f"u{ch}")
                 for ch in range(NCH)]
            bl = [stpool.tile([128, ISH, O], f32, name=f"b{ch}")
                  for ch in range(NCH)]
            cl = [stpool.tile([128, ISH, O], bf16, name=f"c{ch}")
                  for ch in range(NCH)]
            vb = [stpool.tile([128, O, DO], bf16, name=f"vb{ch}")
                  for ch in range(NCH)]

            ar_in = [dram.tile([NCH, 128, O, DO], f32, name=f"arin{t}")
                     for t in range(3)]
            ar_out = [dram.tile([NCH, 128, O, DO], f32, name=f"arout{t}")
                      for t in range(3)]

            # bank-aligned pieces of a duo psum [0,1280): (group, lo, hi)
            duo_pieces = [(0, 0, 512), (0, 512, 640),
                          (1, 640, 1024), (1, 1024, 1280)]

            # ---- phase 1: u_hat + s1 ---------------------------------------
            for ch in range(NCH):
                bsl = slice(ch * BP, ch * BP + BP)
                s1p = psum_s.tile([128, OD], f32, name="s1p")
                for q in range(NQ):
                    for duo in range(2):          # two 2-group duos per quad
                        dp = psum_u.tile([128, 1280], f32, name="dp")
                        for (gg, lo, hi) in duo_pieces:
                            sub = 2 * duo + gg    # group index within quad
                            lhsT = xT[q][32 * sub:32 * sub + 32, bsl]
                            nc.tensor.matmul(
                                dp[:, lo:hi],
                                lhsT,
                                Wbd[q][32 * sub:32 * sub + 32,
                                       lo - 640 * gg:hi - 640 * gg],
                                start=True, stop=True,
                                tile_position=(32 * sub, 0),
                            )
                        # evacuate duo -> u slice (8 i's, contiguous)
                        i0 = 16 * q + 8 * duo
                        dst = u[ch][:, i0:i0 + 8, :, :]
                        src = dp.rearrange("p (i o d) -> p i o d", i=8, o=O)
                        if (2 * q + duo) % 2 == 0:
                            nc.vector.tensor_copy(out=dst, in_=src)
                        else:
                            nc.scalar.copy(out=dst, in_=src)
                    # s1 partial: K=128 (sums the block's 16 i's — wanted),
                    # full-array matmul so the accumulation chain stays at a
                    # single tile position (mixed-position chains wedge HW).
                    nc.tensor.matmul(
                        s1p[:], xT[q][:, bsl], Wk[q][:],
                        start=(q == 0), stop=(q == NQ - 1),
                    )
                s1 = small.tile([128, O, DO], f32, name="s1")
                nc.scalar.mul(out=s1[:].rearrange("p o d -> p (o d)"),
                              in_=s1p[:], mul=0.1)
                nc.sync.dma_start(ar_in[0][ch], s1[:])

            # ---- helpers ----------------------------------------------------
            def squash(it, ch):
                """AllReduced s -> v (f32 in vb-bf16 + returns f32 tile)."""
                s = small.tile([128, O, DO], f32, name="ssum")
                nc.sync.dma_start(s[:], ar_out[it][ch])
                sq = small.tile([128, O, DO], f32, name="sq")
                nc.vector.tensor_mul(out=sq[:], in0=s[:], in1=s[:])
                n2 = small.tile([128, O], f32, name="n2")
                nc.vector.tensor_reduce(n2[:], sq[:], X, Alu.add)
                nrm = small.tile([128, O], f32, name="nrm")
                nc.scalar.activation(nrm[:], n2[:], Act.Sqrt)
                t1 = small.tile([128, O], f32, name="t1")
                nc.vector.tensor_scalar_add(t1[:], n2[:], 1.0)
                t2 = small.tile([128, O], f32, name="t2")
                nc.vector.tensor_scalar_add(t2[:], nrm[:], 1e-8)
                den = small.tile([128, O], f32, name="den")
                nc.vector.tensor_mul(out=den[:], in0=t1[:], in1=t2[:])
                rden = small.tile([128, O], f32, name="rden")
                nc.vector.reciprocal(out=rden[:], in_=den[:])
                scl = small.tile([128, O], f32, name="scl")
                nc.vector.tensor_mul(out=scl[:], in0=n2[:], in1=rden[:])
                vf = small.tile([128, O, DO], f32, name="vf")
                nc.vector.tensor_tensor(
                    vf[:], s[:],
                    scl[:, :, None].to_broadcast([128, O, DO]), Alu.mult)
                nc.vector.tensor_copy(out=vb[ch][:], in_=vf[:])
                return vf

            def uv_pass(ch, first):
                """b += u . v  (contract over d); first iter writes b."""
                for q in range(NQ):
                    isl = slice(16 * q, 16 * q + 16)
                    tmp = tmppool.tile([128, 16, O, DO], f32, name="uvtmp")
                    nc.vector.tensor_tensor(
                        tmp[:], u[ch][:, isl, :, :],
                        vb[ch][:, None, :, :].to_broadcast([128, 16, O, DO]),
                        Alu.mult)
                    if first:
                        nc.vector.tensor_reduce(
                            bl[ch][:, isl, :], tmp[:], X, Alu.add)
                    else:
                        uvb = small.tile([128, 16, O], f32, name="uvb")
                        nc.vector.tensor_reduce(uvb[:], tmp[:], X, Alu.add)
                        nc.vector.tensor_add(
                            out=bl[ch][:, isl, :],
                            in0=bl[ch][:, isl, :], in1=uvb[:])

            def c_pass(ch):
                """c = softmax_o(b)   (b is small enough to be exp-safe)."""
                e = small.tile([128, ISH, O], bf16, name="e")
                nc.scalar.activation(e[:], bl[ch][:], Act.Exp)
                Z = small.tile([128, ISH], f32, name="Z")
                nc.vector.tensor_reduce(Z[:], e[:], X, Alu.add)
                rZ = small.tile([128, ISH], bf16, name="rZ")
                with nc.allow_low_precision(reason="softmax denom, |b| << 1"):
                    nc.vector.reciprocal(out=rZ[:], in_=Z[:])
                nc.vector.tensor_tensor(
                    cl[ch][:], e[:],
                    rZ[:, :, None].to_broadcast([128, ISH, O]), Alu.mult)

            def s_pass(it, ch):
                """s_partial = sum_i c*u -> ar_in[it][ch]."""
                sacc = small.tile([128, O, DO], f32, name="sacc")
                for q in range(NQ):
                    isl = slice(16 * q, 16 * q + 16)
                    tmp2 = tmppool.tile([128, O, DO, 16], f32, name="stmp")
                    nc.vector.tensor_tensor(
                        tmp2.rearrange("p o d i -> p i o d"),
                        u[ch][:, isl, :, :],
                        cl[ch][:, isl, :, None].to_broadcast(
                            [128, 16, O, DO]),
                        Alu.mult)
                    if q == 0:
                        nc.vector.tensor_reduce(sacc[:], tmp2[:], X, Alu.add)
                    else:
                        sblk = small.tile([128, O, DO], f32, name="sblk")
                        nc.vector.tensor_reduce(sblk[:], tmp2[:], X, Alu.add)
                        nc.vector.tensor_add(out=sacc[:], in0=sacc[:],
                                             in1=sblk[:])
                nc.sync.dma_start(ar_in[it][ch], sacc[:])

            def allreduce(it):
                nc.gpsimd.collective_compute(
                    "AllReduce", Alu.add,
                    replica_groups=[list(range(N_CORES))],
                    ins=[ar_in[it].opt()], outs=[ar_out[it].opt()])

            # ---- routing ----------------------------------------------------
            allreduce(0)                      # iteration 1: s1
            for ch in range(NCH):
                squash(0, ch)                 # v1
                uv_pass(ch, first=True)       # b = u.v1

            for ch in range(NCH):             # iteration 2
                c_pass(ch)
                s_pass(1, ch)
            allreduce(1)
            for ch in range(NCH):
                squash(1, ch)                 # v2
                uv_pass(ch, first=False)      # b += u.v2

            for ch in range(NCH):             # iteration 3 (final)
                c_pass(ch)
                s_pass(2, ch)
            allreduce(2)
            for ch in range(NCH):
                vf = squash(2, ch)            # v3 = output
                nc.sync.dma_start(y_d[ch * BP:ch * BP + BP, :, :], vf[:])

    nc.compile()
    return nc


def _prep_inputs(x, weight):
    """Host-side resharding: returns per-core input dicts."""
    import ml_dtypes

    bf = ml_dtypes.bfloat16
    x = np.asarray(x, dtype=np.float32)
    w = np.asarray(weight, dtype=np.float32)[0]      # [O, I, DO, DI]
    in_maps = []
    for c in range(N_CORES):
        isl = slice(c * ISH, (c + 1) * ISH)
        xs = x[:, isl, :]                            # [B, 144, 8]
        # xT[q, (si16, di8), b]
        xT = xs.reshape(B, NQ, 16, DI).transpose(1, 2, 3, 0).reshape(
            NQ, 128, B)
        ws = w[:, isl, :, :]                         # [O, 144, DO, DI]
        # Wg[g, j, di, (o d)]
        wg = ws.transpose(1, 3, 0, 2).reshape(ISH // 4, 4, DI, OD)
        wk = wg.reshape(ISH // 4, 32, OD)            # dense K=32 blocks
        wbd = np.zeros((ISH // 4, 32, 4 * OD), dtype=np.float32)
        for j in range(4):
            wbd[:, 8 * j:8 * j + 8, OD * j:OD * j + OD] = wg[:, j]
        in_maps.append({
            "xT": np.ascontiguousarray(xT.astype(bf)),
            "Wbd": np.ascontiguousarray(
                wbd.reshape(NQ, 128, 4 * OD).astype(bf)),
            "Wk": np.ascontiguousarray(wk.reshape(NQ, 128, OD).astype(bf)),
        })
    return in_maps


def kernel(x, weight):
    from concourse.bass_utils import run_bass_kernel_spmd

    if "nc" not in _cached:
        _cached["nc"] = _build()
    in_maps = _prep_inputs(x, weight)
    res = run_bass_kernel_spmd(
        _cached["nc"], in_maps, core_ids=list(range(N_CORES)))
    return res.results[0]["y"].astype(np.float32)
